# revision 8
# baseline (speedup 1.0000x reference)
"""Trainium2 Bass kernel for a ResNet bottleneck block (training-mode BN).

Computes, for x of shape (64, 1024, 14, 14):
    y1 = relu(bn(conv1x1(x, w1)))        # 1024 -> 256
    y2 = relu(bn(conv3x3(y1, w2)))       # 256 -> 256, pad 1
    z3 = bn(conv1x1(y2, w3))             # 256 -> 1024
    out = relu(x + softplus(residual_scale) * z3)

BN is training-mode: per-channel mean/var over (N, H, W) of the full batch.

Sharding: data-parallel over the batch dim, 8 images per NeuronCore. Exact
global BN statistics via tiny AllGathers of per-core (count, mean, var)
triples, combined with bn_aggr (exact for equal per-core counts).

Layout/overlap decisions (v2):
  - All DRAM I/O is partition-major with one contiguous run per partition
    (input x: 25KB/partition, per-chunk output: 6.3KB/partition) so DMAs
    run at full HBM bandwidth; the host does the cheap transposes.
  - A dependency-free dummy AllGather triggers first, so the runtime's
    first-collective barrier (cross-core launch skew, ~27us) is absorbed
    under the input DMA + conv1 instead of delaying the bn1 exchange.
  - Stat exchanges are split (bn2 per chunk, bn3 in two halves) so most
    of the collective latency hides under the conv that produces the
    later chunks.
  - z tensors are stored bf16; bn_stats reads them from SBUF so PSUM
    banks are freed by the ACT eviction alone.
  - Exchange gather DMAs ride the tensor queue (idle between convs),
    stat uploads + even-chunk outputs ride sync, odd-chunk outputs ride
    scalar: no FIFO ever blocks a ready instruction behind a waiting one.
  - The residual tail splits work across DVE/ACT/GpSimd and streams each
    chunk's output DMA as soon as it is ready.
"""

import os
import numpy as np
import ml_dtypes
from contextlib import ExitStack

import concourse.bass as bass
import concourse.bacc as bacc
import concourse.mybir as mybir
import concourse.tile as tile
from concourse.bass_utils import run_bass_kernel_spmd

F32 = mybir.dt.float32
BF16 = mybir.dt.bfloat16
AX = mybir.AxisListType
ALU = mybir.AluOpType
ACTF = mybir.ActivationFunctionType

N_CORES = 8
N, CIN, H, W = 64, 1024, 14, 14
P = 256
COUT = 1024
NL = N // N_CORES          # images per core (8)
HW = H * W                 # 196
F = NL * HW                # free positions per core (1568)
FT = 4                     # free-dim tiles
FTS = F // FT              # 392 positions per tile (= 2 images)
IPT = NL // FT             # images per free tile (2)
CI_CH = CIN // 128         # 8
P_CH = P // 128            # 2
CO_CH = COUT // 128        # 8
EPS = 1e-5
PAD = 16                   # padded spatial stride (16x16 per image)
SG = 4                     # bn_stats groups per channel chunk (F = SG*392)


def _emit_ag(nc, tc, ctx, name, st_ap, n_ch):
    """AllGather local [128, n_ch, 3] (count, mean, var) triples.
    Upload on sync queue, trigger from gpsimd. Returns (cc_out, gather_fn)
    where gather_fn(queue_engine) DMAs the gathered triples to SBUF and
    returns the [128, N_CORES, n_ch, 3] tile."""
    dram = ctx.enter_context(tc.tile_pool(name=f"{name}_dram", bufs=1, space="DRAM"))
    sb = ctx.enter_context(tc.tile_pool(name=f"{name}_sb", bufs=1))
    cc_in = dram.tile([128, 3 * n_ch], F32, name=f"{name}_in")
    cc_out = dram.tile([N_CORES, 128, 3 * n_ch], F32, addr_space="Shared",
                       name=f"{name}_out")
    nc.sync.dma_start(cc_in[:], st_ap.rearrange("p c t -> p (c t)"))
    nc.gpsimd.collective_compute(
        "AllGather",
        ALU.bypass,
        replica_groups=[list(range(N_CORES))],
        ins=[cc_in.opt()],
        outs=[cc_out.opt()],
    )

    def gather(eng):
        gath = sb.tile([128, N_CORES, n_ch, 3], F32, name=f"{name}_gath")
        eng.dma_start(
            gath.rearrange("p r c t -> p r (c t)"),
            cc_out.rearrange("r p x -> p r x"))
        return gath

    return gather


def _emit_bn_params(nc, tc, ctx, name, gath, gamma, beta, n_ch, epst):
    """Combine gathered triples and compute per-channel affine (a, b):
    bn(z) = a * z + b.  a = gamma * rsqrt(var + eps), b = beta - a * mean."""
    sb = ctx.enter_context(tc.tile_pool(name=f"{name}_bn", bufs=1))
    g2 = sb.tile([128, n_ch, N_CORES, 3], F32, name=f"{name}_g2")
    nc.vector.tensor_copy(g2[:], gath.rearrange("p r c t -> p c r t"))
    mv = sb.tile([128, n_ch, 2], F32, name=f"{name}_mv")
    for c in range(n_ch):
        nc.vector.bn_aggr(mv[:, c], g2[:, c].rearrange("p r t -> p (r t)"))
    std = sb.tile([128, n_ch], F32, name=f"{name}_std")
    nc.scalar.activation(std[:], mv[:, :, 1], ACTF.Sqrt, bias=epst[:, 0:1])
    ar = sb.tile([128, n_ch], F32, name=f"{name}_ar")
    nc.vector.reciprocal(ar[:], std[:])
    a = sb.tile([128, n_ch], F32, name=f"{name}_a")
    nc.vector.tensor_mul(a[:], ar[:], gamma)
    am = sb.tile([128, n_ch], F32, name=f"{name}_am")
    nc.vector.tensor_mul(am[:], a[:], mv[:, :, 0])
    b = sb.tile([128, n_ch], F32, name=f"{name}_b")
    nc.vector.scalar_tensor_tensor(
        b[:], am[:], -1.0, beta, op0=ALU.mult, op1=ALU.add
    )
    return a, b


def build():
    nc = bacc.Bacc("TRN2", target_bir_lowering=False, debug=False,
                   num_devices=N_CORES)

    # ---- I/O: all partition-major, contiguous per partition -------------
    xb_d = nc.dram_tensor("xb16", [128, CI_CH, F], BF16, kind="ExternalInput")
    w1_d = nc.dram_tensor("w1t", [128, CI_CH, P], BF16, kind="ExternalInput")
    w2_d = nc.dram_tensor("w2t", [128, P_CH, 9, P], BF16, kind="ExternalInput")
    w3_d = nc.dram_tensor("w3t", [128, P_CH, COUT], BF16, kind="ExternalInput")
    gb1_d = nc.dram_tensor("gb1", [2, 128, P_CH], F32, kind="ExternalInput")
    gb2_d = nc.dram_tensor("gb2", [2, 128, P_CH], F32, kind="ExternalInput")
    gb3_d = nc.dram_tensor("gb3", [2, 128, CO_CH], F32, kind="ExternalInput")
    out_d = nc.dram_tensor("out", [CO_CH, 128, F], F32, kind="ExternalOutput")

    with tile.TileContext(nc) as tc, ExitStack() as ctx:
        consts0 = ctx.enter_context(tc.tile_pool(name="consts0", bufs=1))
        consts = ctx.enter_context(tc.tile_pool(name="consts", bufs=1))
        xpool = ctx.enter_context(tc.tile_pool(name="xpool", bufs=1))
        actp = ctx.enter_context(tc.tile_pool(name="actp", bufs=1))
        statp = ctx.enter_context(tc.tile_pool(name="statp", bufs=1))
        scrp = ctx.enter_context(tc.tile_pool(name="scrp", bufs=2))
        psum = ctx.enter_context(tc.tile_pool(name="psum", bufs=4, space="PSUM"))

        # ---- dummy collective, zero dependencies: absorbs the runtime's
        # first-collective barrier (cross-core launch skew) immediately.
        dpool = ctx.enter_context(
            tc.tile_pool(name="dummy_dram", bufs=1, space="DRAM"))
        dum_in = dpool.tile([128, 2], F32, name="dummy_in")
        dum_out = dpool.tile([N_CORES, 128, 2], F32, addr_space="Shared",
                             name="dummy_out")
        dscr = consts0.tile([128, 2], F32, name="dummy_scr")
        nc.vector.memset(dscr[:], 0.0)
        nc.sync.dma_start(dum_in[:], dscr[:])
        nc.gpsimd.collective_compute(
            "AllGather",
            ALU.bypass,
            replica_groups=[list(range(N_CORES))],
            ins=[dum_in.opt()],
            outs=[dum_out.opt()],
        )

        # ---- input DMAs, all on the sync queue in priority order:
        # w1, x, bn affine params, then the later-stage weights.
        w1sb_t = consts.tile([128, CI_CH, P], BF16, name="w1sb")
        nc.sync.dma_start(w1sb_t[:], w1_d[:])
        w1sb = [w1sb_t[:, c] for c in range(CI_CH)]
        xb_t = xpool.tile([128, CI_CH, F], BF16, name="xb")
        XH = CI_CH // 2
        nc.sync.dma_start(xb_t[:, :XH], xb_d[:, :XH])
        nc.sync.dma_start(xb_t[:, XH:], xb_d[:, XH:])
        xb = [xb_t[:, c] for c in range(CI_CH)]

        g1 = consts.tile([128, P_CH], F32, name="g1")
        be1 = consts.tile([128, P_CH], F32, name="be1")
        g2 = consts.tile([128, P_CH], F32, name="g2")
        be2 = consts.tile([128, P_CH], F32, name="be2")
        g3 = consts.tile([128, CO_CH], F32, name="g3")
        be3 = consts.tile([128, CO_CH], F32, name="be3")
        for t, d in ((g1, gb1_d), (g2, gb2_d), (g3, gb3_d)):
            nc.sync.dma_start(t[:], d[0])
        for t, d in ((be1, gb1_d), (be2, gb2_d), (be3, gb3_d)):
            nc.sync.dma_start(t[:], d[1])

        w2sb_t = consts.tile([128, P_CH, 9, P], BF16, name="w2sb")
        nc.sync.dma_start(w2sb_t[:], w2_d[:])
        w2sb = [w2sb_t[:, c] for c in range(P_CH)]
        w3sb_t = consts.tile([128, P_CH, COUT], BF16, name="w3sb")
        nc.sync.dma_start(w3sb_t[:], w3_d[:])
        w3sb = [w3sb_t[:, c] for c in range(P_CH)]

        epst = consts.tile([128, 1], F32, name="epst")
        nc.vector.memset(epst[:], EPS)

        # padded bf16 activations for the 3x3 conv: [128, NL, 16, 16]
        y1p = [actp.tile([128, NL, PAD, PAD], BF16, name=f"y1p{c}")
               for c in range(P_CH)]
        for c in range(P_CH):
            nc.vector.memset(y1p[c][:], 0)

        z1 = [actp.tile([128, F], BF16, name=f"z1_{c}") for c in range(P_CH)]
        z2 = [actp.tile([128, F], BF16, name=f"z2_{c}") for c in range(P_CH)]
        y2 = [actp.tile([128, F], BF16, name=f"y2_{c}") for c in range(P_CH)]
        z3 = [actp.tile([128, F], BF16, name=f"z3_{c}") for c in range(CO_CH)]

        # local-stat staging: count planes pre-set (equal counts per core)
        bs1 = statp.tile([128, P_CH, SG, 6], F32, name="bs1")
        st1 = statp.tile([128, P_CH, 3], F32, name="st1")
        bs2 = statp.tile([128, P_CH, SG, 6], F32, name="bs2")
        st2 = statp.tile([128, P_CH, 3], F32, name="st2")
        bs3 = statp.tile([128, CO_CH, SG, 6], F32, name="bs3")
        st3 = statp.tile([128, CO_CH, 3], F32, name="st3")
        for st in (st1, st2, st3):
            nc.vector.memset(st[:, :, 0], 1.0)

        # ================= stage A: conv1 (1x1, 1024 -> 256) =============
        # accumulation groups sequential per PSUM region (group-outer);
        # PSUM tiles are double-bank [128,2,512]: two groups accumulate
        # into halves, one ACT evicts both (fp32 PSUM -> bf16 SBUF).
        for co in range(P_CH):
            for fp in range(2):
                pt = psum.tile([128, 2, 512], F32, name="pt", tag="pt")
                for half in range(2):
                    ft = fp * 2 + half
                    for ci in range(CI_CH):
                        nc.tensor.matmul(
                            pt[:, half, :FTS],
                            w1sb[ci][:, co * 128:(co + 1) * 128],
                            xb[ci][:, ft * FTS:(ft + 1) * FTS],
                            start=(ci == 0),
                            stop=(ci == CI_CH - 1),
                        )
                nc.scalar.copy(
                    z1[co][:, fp * 2 * FTS:(fp + 1) * 2 * FTS]
                        .rearrange("p (a b) -> p a b", a=2),
                    pt[:, :, :FTS],
                )
            # one-pass stats on the bf16 SBUF copy (PSUM freed by ACT alone)
            for g in range(SG):
                nc.vector.bn_stats(
                    bs1[:, co, g], z1[co][:, g * FTS:(g + 1) * FTS])
            nc.vector.bn_aggr(
                st1[:, co, 1:3], bs1[:, co].rearrange("p g s -> p (g s)"))
        gather1 = _emit_ag(nc, tc, ctx, "bn1", st1[:], P_CH)
        gath1 = gather1(nc.sync)
        a1, b1 = _emit_bn_params(nc, tc, ctx, "bn1", gath1, g1[:], be1[:],
                                 P_CH, epst)

        for q in range(4):
            for c in range(P_CH):
                nc.scalar.activation(
                    y1p[c][:, q * IPT:(q + 1) * IPT, 1:1 + H, 1:1 + W],
                    z1[c].rearrange("p (n h w) -> p n h w", n=NL, h=H, w=W)
                        [:, q * IPT:(q + 1) * IPT],
                    ACTF.Relu,
                    bias=b1[:, c:c + 1],
                    scale=a1[:, c:c + 1],
                )

        # ================= stage B: conv2 (3x3, 256 -> 256) ==============
        # per-chunk stat exchange: chunk 0's AllGather hides under chunk
        # 1's matmuls.
        gathers2 = []
        for co in range(P_CH):
            for fp in range(2):
                pt = psum.tile([128, 2, 512], F32, name="pt", tag="pt")
                for half in range(2):
                    ft = fp * 2 + half
                    for ci in range(P_CH):
                        for tap in range(9):
                            ky, kx = divmod(tap, 3)
                            nc.tensor.matmul(
                                pt[:, half, :FTS],
                                w2sb[ci][:, tap, co * 128:(co + 1) * 128],
                                y1p[ci][:, ft * IPT:(ft + 1) * IPT,
                                        ky:ky + H, kx:kx + W],
                                start=(ci == 0 and tap == 0),
                                stop=(ci == P_CH - 1 and tap == 8),
                            )
                nc.scalar.copy(
                    z2[co][:, fp * 2 * FTS:(fp + 1) * 2 * FTS]
                        .rearrange("p (a b) -> p a b", a=2),
                    pt[:, :, :FTS],
                )
            for g in range(SG):
                nc.vector.bn_stats(
                    bs2[:, co, g], z2[co][:, g * FTS:(g + 1) * FTS])
            nc.vector.bn_aggr(
                st2[:, co, 1:3], bs2[:, co].rearrange("p g s -> p (g s)"))
            gathers2.append(
                _emit_ag(nc, tc, ctx, f"bn2_{co}", st2[:, co:co + 1], 1))
        ab2 = []
        for co in range(P_CH):
            gath = gathers2[co](nc.sync)
            ab2.append(_emit_bn_params(nc, tc, ctx, f"bn2_{co}", gath,
                                       g2[:, co:co + 1], be2[:, co:co + 1],
                                       1, epst))
        for c in range(P_CH):
            a2, b2 = ab2[c]
            for q in range(4):
                nc.scalar.activation(
                    y2[c][:, q * FTS:(q + 1) * FTS],
                    z2[c][:, q * FTS:(q + 1) * FTS], ACTF.Relu,
                    bias=b2[:, 0:1], scale=a2[:, 0:1],
                )

        # ================= stage C: conv3 (1x1, 256 -> 1024) =============
        # stat exchange in two halves: the first (chunks 0-3) triggers at
        # conv3's midpoint and hides under the second half's matmuls.
        for co in range(CO_CH):
            for fp in range(2):
                pt = psum.tile([128, 2, 512], F32, name="pt", tag="pt")
                for half in range(2):
                    ft = fp * 2 + half
                    for ci in range(P_CH):
                        nc.tensor.matmul(
                            pt[:, half, :FTS],
                            w3sb[ci][:, co * 128:(co + 1) * 128],
                            y2[ci][:, ft * FTS:(ft + 1) * FTS],
                            start=(ci == 0),
                            stop=(ci == P_CH - 1),
                        )
                nc.scalar.copy(
                    z3[co][:, fp * 2 * FTS:(fp + 1) * 2 * FTS]
                        .rearrange("p (a b) -> p a b", a=2),
                    pt[:, :, :FTS],
                )
            for g in range(SG):
                nc.vector.bn_stats(
                    bs3[:, co, g], z3[co][:, g * FTS:(g + 1) * FTS])
            nc.vector.bn_aggr(
                st3[:, co, 1:3], bs3[:, co].rearrange("p g s -> p (g s)"))
            if co == 3:
                gather3a = _emit_ag(nc, tc, ctx, "bn3a", st3[:, 0:4], 4)
        gather3b = _emit_ag(nc, tc, ctx, "bn3b", st3[:, 4:8], 4)

        gath3a = gather3a(nc.sync)
        ab3a = _emit_bn_params(nc, tc, ctx, "bn3a", gath3a, g3[:, 0:4],
                               be3[:, 0:4], 4, epst)

        # tail: out = relu((a3*z3 + b3) + x), output DMA per chunk as soon
        # as it is ready.  DVE: all scale ops + half the adds + 2 relus;
        # GpSimd: half the adds; ACT: 6 relus.  Outputs alternate the
        # sync / scalar HW DGE queues.
        outf = [actp.tile([128, F], F32, name=f"outf{c}") for c in range(CO_CH)]
        tts = [scrp.tile([128, F], BF16, name=f"t{c}", tag=f"t{c % 4}")
               for c in range(CO_CH)]

        def tail_chunk(co, a3, b3, ci):
            t = tts[co]
            nc.vector.tensor_scalar(
                t[:], z3[co][:], a3[:, ci:ci + 1], b3[:, ci:ci + 1],
                op0=ALU.mult, op1=ALU.add,
            )
            if co % 2 == 0:
                nc.vector.tensor_add(t[:], t[:], xb[co][:])
            else:
                nc.gpsimd.tensor_add(t[:], t[:], xb[co][:])
            if co < 6:
                nc.scalar.activation(outf[co][:], t[:], ACTF.Relu)
            else:
                nc.vector.tensor_scalar_max(outf[co][:], t[:], 0.0)
            deng = nc.sync if co % 2 == 0 else nc.scalar
            deng.dma_start(out_d[co], outf[co][:])

        for co in range(4):
            tail_chunk(co, ab3a[0], ab3a[1], co)
        gath3b = gather3b(nc.sync)
        ab3b = _emit_bn_params(nc, tc, ctx, "bn3b", gath3b, g3[:, 4:8],
                               be3[:, 4:8], 4, epst)
        for co in range(4, 8):
            tail_chunk(co, ab3b[0], ab3b[1], co - 4)
    nc.compile()
    return nc


_NC_CACHE = None


def _get_nc():
    global _NC_CACHE
    if _NC_CACHE is None:
        _NC_CACHE = build()
    return _NC_CACHE


def _prep_host(w1, w2, w3, g1, be1, g2, be2, g3, be3, residual_scale):
    bf = ml_dtypes.bfloat16
    # conv weights, pre-transposed to partition-major [p, ci_chunk, ...]
    w1t = np.ascontiguousarray(
        w1.reshape(P, CIN).T.astype(bf).reshape(CI_CH, 128, P)
        .transpose(1, 0, 2))
    w2t = np.ascontiguousarray(
        w2.transpose(1, 2, 3, 0).astype(bf).reshape(P_CH, 128, 9, P)
        .transpose(1, 0, 2, 3))
    w3t = np.ascontiguousarray(
        w3.reshape(COUT, P).T.astype(bf).reshape(P_CH, 128, COUT)
        .transpose(1, 0, 2))
    s = np.float32(np.log1p(np.exp(np.float64(residual_scale[0]))))
    gb1 = np.ascontiguousarray(np.stack([g1, be1]).astype(np.float32)
                               .reshape(2, P_CH, 128).transpose(0, 2, 1))
    gb2 = np.ascontiguousarray(np.stack([g2, be2]).astype(np.float32)
                               .reshape(2, P_CH, 128).transpose(0, 2, 1))
    gb3 = np.ascontiguousarray((np.stack([g3, be3]) * s).astype(np.float32)
                               .reshape(2, CO_CH, 128).transpose(0, 2, 1))
    return w1t, w2t, w3t, gb1, gb2, gb3


def prepare_in_maps(inputs):
    x = np.asarray(inputs["x"], dtype=np.float32)
    w1t, w2t, w3t, gb1, gb2, gb3 = _prep_host(
        np.asarray(inputs["w1"], np.float32), np.asarray(inputs["w2"], np.float32),
        np.asarray(inputs["w3"], np.float32), np.asarray(inputs["g1"], np.float32),
        np.asarray(inputs["be1"], np.float32), np.asarray(inputs["g2"], np.float32),
        np.asarray(inputs["be2"], np.float32), np.asarray(inputs["g3"], np.float32),
        np.asarray(inputs["be3"], np.float32),
        np.asarray(inputs["residual_scale"], np.float32),
    )
    in_maps = []
    for c in range(N_CORES):
        shard = x[c * NL:(c + 1) * NL].reshape(NL, CIN, HW)
        # [n, (cc p), f] -> [p, cc, n, f] (one contiguous run per partition)
        xb16 = np.ascontiguousarray(
            shard.reshape(NL, CI_CH, 128, HW).transpose(2, 1, 0, 3)
            .astype(ml_dtypes.bfloat16)).reshape(128, CI_CH, F)
        in_maps.append({
            "xb16": xb16, "w1t": w1t, "w2t": w2t, "w3t": w3t,
            "gb1": gb1, "gb2": gb2, "gb3": gb3,
        })
    return in_maps


def kernel(**inputs):
    in_maps = prepare_in_maps(inputs)
    nc = _get_nc()
    trace = bool(int(os.environ.get("KERNEL_PROFILE", "0")))
    try:
        res = run_bass_kernel_spmd(nc, in_maps, list(range(N_CORES)), trace=trace)
    except ModuleNotFoundError:
        # axon NTFF profile hook unavailable in this container
        res = run_bass_kernel_spmd(nc, in_maps, list(range(N_CORES)), trace=False)
    if trace:
        kernel.last_exec_time_ns = getattr(res, "exec_time_ns", None)
        kernel.last_profile = res
    # out core result: [co_chunk, p, n*f] -> [n, (co_chunk p), f]
    outs = []
    for c in range(N_CORES):
        o = res.results[c]["out"].reshape(CO_CH, 128, NL, HW)
        outs.append(o.transpose(2, 0, 1, 3).reshape(NL, CIN, HW))
    return np.concatenate(outs, axis=0).reshape(N, CIN, H, W)
